# revision 2
# baseline (speedup 1.0000x reference)
"""Mistral attention (B=2, S=2048, H=4096, 32 q heads / 8 kv heads, rope) on
8 Trainium2 NeuronCores.

Sharding: DP=2 over batch x TP=4 over head groups. Core c handles batch
c//4 and q-heads 8g..8g+7 (kv heads 2g, 2g+1) where g = c%4. Attention is
fully local per core; each core produces a partial o_proj output
[2048, 4096] (fp16) and the host sums the four TP partials per batch in
fp32.

Device pipeline (per core), everything fp16 except PSUM/softmax stats:
  - Phase 1 (projections): lhsT = host-packed contiguous weight tiles,
    rhs = host-pretransposed X^T strips. RoPE applied in fp16 on DVE after
    an ACT fp32->fp16 copy of the PSUM projection result. PE warm-up
    matmuls cover the initial DMA latency so the p-state ramp finishes
    before real work.
  - Phase 2+3 (attention + o_proj, interleaved): scores transposed S^T
    [k, q]; exp on ACT (scale=1/sqrt(d), bias=-2 for fp16 range); causal
    handling = free-dim narrowing per diagonal tile + one [128,128]
    triangle mask; softmax denominator accumulated in fp16 on DVE and
    partition-reduced with gpsimd.partition_all_reduce (no PE matmuls, no
    extra PSUM banks); PV accumulates out^T [d, q] in PSUM. o_proj matmuls
    of the previous query strip are interleaved between PV matmuls as PE
    filler so the ACT-paced exp pipeline never idles the PE.
"""
import sys

if "/opt/trn_rl_repo" not in sys.path:
    sys.path.insert(0, "/opt/trn_rl_repo")

import numpy as np

S = 2048          # sequence length per core
H = 4096          # hidden
D = 128           # head dim
TP = 4            # head-group shards
DP = 2            # batch shards
NHC = 8           # q heads per core
NKVC = 2          # kv heads per core
NPROJ = NHC + NKVC
KO = H // 128     # 32 contraction tiles for projections
NT = S // 128     # 16 t tiles
SCALE = 1.0 / np.sqrt(D)
# Folded into exp; cancels exactly in the softmax normalization. Near-diagonal
# scores reach ~16 after the 1/sqrt(d) scale (q_t and k_t share x_t), so the
# bias must keep exp(smax + bias) under fp16's 65504: -10 handles smax <= 21.
EXP_BIAS = -10.0
ROPE_THETA = 10000.0

_CACHE = {}


def _build_nc():
    import concourse.mybir as mybir
    import concourse.tile as tile
    from concourse import bacc, bass_isa

    DT = mybir.dt
    ACTF = mybir.ActivationFunctionType
    nc = bacc.Bacc(None, target_bir_lowering=False)

    xT = nc.dram_tensor("xT", [128, KO, S], DT.float16, kind="ExternalInput")
    wqk = nc.dram_tensor("wqk", [NPROJ, 128, KO, 128], DT.float16,
                         kind="ExternalInput")
    wvT = nc.dram_tensor("wvT", [128, KO, NKVC * 128], DT.float16,
                         kind="ExternalInput")
    woT = nc.dram_tensor("woT", [128, NHC, H], DT.float16, kind="ExternalInput")
    cosT = nc.dram_tensor("cosT", [128, S], DT.float16, kind="ExternalInput")
    sinT = nc.dram_tensor("sinT", [128, S], DT.float16, kind="ExternalInput")
    trim = nc.dram_tensor("trim", [128, 128], DT.float16, kind="ExternalInput")
    outp = nc.dram_tensor("outp", [S, H], DT.float16, kind="ExternalOutput")

    with tile.TileContext(nc) as tc:
        with tc.tile_pool(name="persist", bufs=1) as persist:
            qT = persist.tile([128, NHC, S], DT.float16)    # [d, head, t]
            kT = persist.tile([128, NKVC, S], DT.float16)   # [d, kv, t]
            vsb = persist.tile([128, NT, NKVC * 128], DT.float16)
            biasc = persist.tile([128, 1], DT.float32)
            nc.vector.memset(biasc[:], EXP_BIAS)
            trimask = persist.tile([128, 128], DT.float16)
            nc.sync.dma_start(trimask[:], trim[:])
            wu = persist.tile([128, 512], DT.float16)
            nc.vector.memset(wu[:], 0.0)

            # PE p-state warm-up: dummy matmuls that run while the first
            # DMAs land, so real matmuls start at full clock.
            with tc.tile_pool(name="pwu", bufs=1, space="PSUM") as pwu:
                wups = pwu.tile([128, 512], DT.float32)
                for _ in range(14):
                    nc.tensor.matmul(wups[:], wu[:, 0:128], wu[:],
                                     start=True, stop=True)

            # ---------------- Phase 1: projections + rope ----------------
            with tc.tile_pool(name="p1c", bufs=1) as p1c, \
                 tc.tile_pool(name="p1x", bufs=2) as p1x, \
                 tc.tile_pool(name="p1w", bufs=3) as p1w, \
                 tc.tile_pool(name="p1ri", bufs=2) as p1ri, \
                 tc.tile_pool(name="p1t", bufs=2) as p1t, \
                 tc.tile_pool(name="p1ps", bufs=3, space="PSUM") as p1ps, \
                 tc.tile_pool(name="p1pv", bufs=2, space="PSUM") as p1pv:
                cos_sb = p1c.tile([128, S], DT.float16)
                sin_sb = p1c.tile([128, S], DT.float16)
                wv_sb = p1c.tile([128, KO, NKVC * 128], DT.float16)

                def rope(dst, ri, t0):
                    # cos/sin halves are identical (emb = concat(freqs,
                    # freqs)), so each product reads the half that matches its
                    # ri operand's base partition (BIR requires equal SBUF
                    # base partitions for DVE inputs).
                    cl = cos_sb[0:64, t0:t0 + 512]
                    sl = sin_sb[0:64, t0:t0 + 512]
                    ch = cos_sb[64:128, t0:t0 + 512]
                    sh = sin_sb[64:128, t0:t0 + 512]
                    t1 = p1t.tile([64, 512], DT.float16, tag="t1")
                    t2 = p1t.tile([64, 512], DT.float16, tag="t2")
                    nc.vector.tensor_mul(t1[:], ri[0:64, :], cl)
                    nc.vector.tensor_mul(t2[:], ri[64:128, :], sh)
                    nc.vector.tensor_sub(dst[0:64, :], t1[:], t2[:])
                    t3 = p1t.tile([64, 512], DT.float16, tag="t3")
                    t4 = p1t.tile([64, 512], DT.float16, tag="t4")
                    nc.vector.tensor_mul(t3[:], ri[64:128, :], ch)
                    nc.vector.tensor_mul(t4[:], ri[0:64, :], sl)
                    nc.vector.tensor_add(dst[64:128, :], t3[:], t4[:])

                for ts in range(4):
                    t0 = ts * 512
                    xs = p1x.tile([128, KO, 512], DT.float16, tag="xs")
                    for c in range(4):
                        nc.sync.dma_start(xs[:, 8 * c:8 * c + 8, :],
                                          xT[:, 8 * c:8 * c + 8, t0:t0 + 512])
                    if ts == 0:
                        nc.sync.dma_start(cos_sb[:], cosT[:])
                        nc.sync.dma_start(sin_sb[:], sinT[:])
                        nc.sync.dma_start(wv_sb[:], wvT[:])
                    for h in range(NPROJ):
                        wt = p1w.tile([128, KO, 128], DT.float16, tag="w")
                        nc.sync.dma_start(wt[:], wqk[h])
                        ps = p1ps.tile([128, 512], DT.float32, tag="p")
                        for ko in range(KO):
                            nc.tensor.matmul(ps[:], wt[:, ko, :], xs[:, ko, :],
                                             start=(ko == 0),
                                             stop=(ko == KO - 1))
                        ri = p1ri.tile([128, 512], DT.float16, tag="ri")
                        nc.scalar.copy(ri[:], ps[:])
                        if h < NHC:
                            rope(qT[:, h, t0:t0 + 512], ri, t0)
                        else:
                            rope(kT[:, h - NHC, t0:t0 + 512], ri, t0)
                    for mt in range(4):
                        psv = p1pv.tile([128, NKVC * 128], DT.float32, tag="pv")
                        for ko in range(KO):
                            nc.tensor.matmul(
                                psv[:], xs[:, ko, mt * 128:(mt + 1) * 128],
                                wv_sb[:, ko, :],
                                start=(ko == 0), stop=(ko == KO - 1))
                        nc.scalar.copy(vsb[:, ts * 4 + mt, :], psv[:])

            # ------- Phases 2+3 interleaved -------
            with tc.tile_pool(name="p2c", bufs=1) as p2c, \
                 tc.tile_pool(name="p2pt", bufs=10) as p2pt, \
                 tc.tile_pool(name="p2acc", bufs=2) as p2acc, \
                 tc.tile_pool(name="p2sum", bufs=2) as p2sum, \
                 tc.tile_pool(name="p2y", bufs=2) as p2y, \
                 tc.tile_pool(name="p3o", bufs=3) as p3o, \
                 tc.tile_pool(name="psS", bufs=3, space="PSUM") as psS, \
                 tc.tile_pool(name="psO", bufs=2, space="PSUM") as psO, \
                 tc.tile_pool(name="p3ps", bufs=3, space="PSUM") as p3ps:
                wo_sb = p2c.tile([128, NHC, H], DT.float16)
                for ntc in range(8):
                    nc.sync.dma_start(
                        wo_sb[:, :, 512 * ntc:512 * ntc + 512],
                        woT[:, :, 512 * ntc:512 * ntc + 512])

                # o_proj filler machinery: one step = one PE matmul of the
                # previous strip's o_proj, emitted between attention matmuls.
                fst = {"st": None, "yT": None, "items": [], "cur": None}

                def filler_step():
                    if fst["cur"] is None:
                        if not fst["items"]:
                            return False
                        tt, n0 = fst["items"].pop(0)
                        ps3 = p3ps.tile([128, 512], DT.float32, tag="p3",
                                        name="ps3")
                        fst["cur"] = [tt, n0, 0, ps3]
                    tt, n0, hh, ps3 = fst["cur"]
                    nc.tensor.matmul(ps3[:],
                                     fst["yT"][:, hh, 128 * tt:128 * tt + 128],
                                     wo_sb[:, hh, n0:n0 + 512],
                                     start=(hh == 0), stop=(hh == NHC - 1))
                    if hh == NHC - 1:
                        ot = p3o.tile([128, 512], DT.float16, tag="ot")
                        nc.scalar.copy(ot[:], ps3[:])
                        row0 = (4 * fst["st"] + tt) * 128
                        nc.sync.dma_start(outp[row0:row0 + 128, n0:n0 + 512],
                                          ot[:])
                        fst["cur"] = None
                    else:
                        fst["cur"][2] = hh + 1
                    return True

                for qj in range(4):
                    q0 = qj * 512
                    nki = 4 * qj + 4
                    yT = p2y.tile([128, NHC, 512], DT.float16, tag="y")
                    for h in range(NHC):
                        kv = h // (NHC // NKVC)
                        acc = p2acc.tile([128, 512], DT.float16, tag="acc")
                        ps_o = psO.tile([128, 512], DT.float32, tag="o")
                        ps_list = {}

                        def emit_S(ki):
                            off = max(0, ki - 4 * qj) * 128
                            ps_s = psS.tile([128, 512], DT.float32, tag="s")
                            nc.tensor.matmul(
                                ps_s[:, :512 - off],
                                kT[:, kv, 128 * ki:128 * ki + 128],
                                qT[:, h, q0 + off:q0 + 512],
                                start=True, stop=True)
                            ps_list[ki] = ps_s

                        for ki in range(min(3, nki)):
                            emit_S(ki)
                        for ki in range(nki):
                            off = max(0, ki - 4 * qj) * 128
                            w_ = 512 - off
                            ps_s = ps_list.pop(ki)
                            pt = p2pt.tile([128, 512], DT.float16, tag="pt")
                            nc.scalar.activation(pt[:, :w_], ps_s[:, :w_],
                                                 ACTF.Exp, scale=SCALE,
                                                 bias=biasc[:])
                            if ki >= 4 * qj:
                                nc.vector.tensor_mul(pt[:, 0:128],
                                                     pt[:, 0:128], trimask[:])
                            if ki == 0:
                                nc.vector.tensor_copy(acc[:], pt[:])
                            else:
                                nc.vector.tensor_add(acc[:, off:],
                                                     acc[:, off:], pt[:, :w_])
                            if ki + 3 < nki:
                                emit_S(ki + 3)
                            nc.tensor.matmul(
                                ps_o[:, off:],
                                vsb[:, ki, 128 * kv:128 * kv + 128],
                                pt[:, :w_],
                                start=(ki == 0), stop=(ki == nki - 1),
                                skip_group_check=True)
                            filler_step()
                            filler_step()
                        sums = p2sum.tile([128, 512], DT.float32, tag="sums")
                        nc.gpsimd.partition_all_reduce(
                            sums[:], acc[:], channels=128,
                            reduce_op=bass_isa.ReduceOp.add)
                        rec = p2sum.tile([128, 512], DT.float32, tag="rec")
                        nc.vector.reciprocal(rec[:], sums[:])
                        nc.vector.tensor_mul(yT[:, h, :], ps_o[:], rec[:])
                    while filler_step():
                        pass
                    fst["st"] = qj
                    fst["yT"] = yT
                    fst["items"] = [(tt, 512 * nt)
                                    for tt in range(4) for nt in range(8)]
                while filler_step():
                    pass
    nc.compile()
    return nc


def _get_nc():
    if "nc" not in _CACHE:
        _CACHE["nc"] = _build_nc()
    return _CACHE["nc"]


def _host_prep(hidden_states, position_ids, wq, wk, wv, wo):
    """Build the 8 per-core input maps (all fp16, contiguity-packed)."""
    F16 = np.float16
    inv_freq = 1.0 / (ROPE_THETA ** (np.arange(0, D, 2, dtype=np.float32) / D))
    p = np.arange(128)[:, None]
    f = np.arange(128)[None, :]
    trim = (p <= f).astype(F16)

    DQ = NHC * D
    DKV = NKVC * D
    wq = np.asarray(wq, dtype=np.float32)
    wk = np.asarray(wk, dtype=np.float32)
    wv = np.asarray(wv, dtype=np.float32)
    wo = np.asarray(wo, dtype=np.float32)

    in_maps = []
    for c in range(8):
        b, g = divmod(c, TP)
        x = np.asarray(hidden_states[b], dtype=np.float32)
        xT = np.ascontiguousarray(
            x.T.reshape(KO, 128, S).transpose(1, 0, 2)).astype(F16)
        wq_g = wq[DQ * g:DQ * (g + 1)].reshape(NHC, 128, KO, 128)
        wk_g = wk[DKV * g:DKV * (g + 1)].reshape(NKVC, 128, KO, 128)
        wqk = np.ascontiguousarray(
            np.concatenate([wq_g, wk_g], axis=0).transpose(0, 3, 2, 1)
        ).astype(F16)
        wvT = np.ascontiguousarray(
            wv[DKV * g:DKV * (g + 1)].reshape(DKV, KO, 128).transpose(2, 1, 0)
        ).astype(F16)
        woT = np.ascontiguousarray(
            wo[:, DQ * g:DQ * (g + 1)].T.reshape(NHC, 128, H).transpose(1, 0, 2)
        ).astype(F16)
        pos = np.asarray(position_ids[b], dtype=np.float32)
        freqs = pos[:, None] * inv_freq[None, :]            # [S, 64]
        emb = np.concatenate([freqs, freqs], axis=1)        # [S, 128]
        cosT = np.ascontiguousarray(np.cos(emb).T).astype(F16)
        sinT = np.ascontiguousarray(np.sin(emb).T).astype(F16)
        in_maps.append({
            "xT": xT, "wqk": wqk, "wvT": wvT, "woT": woT,
            "cosT": cosT, "sinT": sinT, "trim": trim,
        })
    return in_maps


def kernel(hidden_states, position_ids, wq, wk, wv, wo):
    from concourse.bass_utils import run_bass_kernel_spmd

    hidden_states = np.asarray(hidden_states)
    in_maps = _host_prep(hidden_states, position_ids, wq, wk, wv, wo)
    nc = _get_nc()
    res = run_bass_kernel_spmd(nc, in_maps, list(range(8)))
    out = np.zeros((DP, S, H), dtype=np.float32)
    for c in range(8):
        b = c // TP
        out[b] += res.results[c]["outp"].astype(np.float32)
    return out


# revision 3
# speedup vs baseline: 1.0216x; 1.0216x over previous
"""Mistral attention (B=2, S=2048, H=4096, 32 q heads / 8 kv heads, rope) on
8 Trainium2 NeuronCores.

Sharding: DP=2 over batch x TP=4 over head groups. Core c handles batch
c//4 and q-heads 8g..8g+7 (kv heads 2g, 2g+1) where g = c%4. Attention is
fully local per core; each core produces a partial o_proj output
[2048, 4096] (fp16) and the host sums the four TP partials per batch in
fp32.

Device pipeline (per core), everything fp16 except PSUM/softmax stats:
  - Phase 1 (projections): lhsT = host-packed contiguous weight tiles,
    rhs = host-pretransposed X^T strips. RoPE applied in fp16 on DVE after
    an ACT fp32->fp16 copy of the PSUM projection result. PE warm-up
    matmuls cover the initial DMA latency so the p-state ramp finishes
    before real work. The query-strip-0 attention block (which has no
    o_proj filler work of its own) is fused INTO phase 1: one attention
    head is emitted between projection head-groups of strips 1-3, so the
    ACT-paced exp pipeline hides behind projection matmuls.
  - Phase 2+3 (attention strips 1-3 + o_proj, interleaved): scores
    transposed S^T [k, q]; exp on ACT (scale=1/sqrt(d), bias=-10 for fp16
    range; the bias cancels in normalization); causal handling = free-dim
    narrowing per diagonal tile + one [128,128] triangle mask; softmax
    denominator accumulated in fp16 on DVE and partition-reduced with
    gpsimd.partition_all_reduce (no PE matmuls, no extra PSUM banks); PV
    accumulates out^T [d, q] in PSUM. o_proj matmuls of the previous query
    strip are interleaved between PV matmuls as PE filler so the ACT-paced
    exp pipeline never idles the PE.
"""
import sys

if "/opt/trn_rl_repo" not in sys.path:
    sys.path.insert(0, "/opt/trn_rl_repo")

import numpy as np

S = 2048          # sequence length per core
H = 4096          # hidden
D = 128           # head dim
TP = 4            # head-group shards
DP = 2            # batch shards
NHC = 8           # q heads per core
NKVC = 2          # kv heads per core
NPROJ = NHC + NKVC
KO = H // 128     # 32 contraction tiles for projections
NT = S // 128     # 16 t tiles
SCALE = 1.0 / np.sqrt(D)
# Folded into exp; cancels exactly in the softmax normalization. Near-diagonal
# scores reach ~16 after the 1/sqrt(d) scale (q_t and k_t share x_t), so the
# bias must keep exp(smax + bias) under fp16's 65504: -10 handles smax <= 21.
EXP_BIAS = -10.0
ROPE_THETA = 10000.0

_CACHE = {}


def _build_nc():
    import concourse.mybir as mybir
    import concourse.tile as tile
    from concourse import bacc, bass_isa

    DT = mybir.dt
    ACTF = mybir.ActivationFunctionType
    nc = bacc.Bacc(None, target_bir_lowering=False)

    xT = nc.dram_tensor("xT", [128, KO, S], DT.float16, kind="ExternalInput")
    wqk = nc.dram_tensor("wqk", [NPROJ, 128, KO, 128], DT.float16,
                         kind="ExternalInput")
    wvT = nc.dram_tensor("wvT", [128, KO, NKVC * 128], DT.float16,
                         kind="ExternalInput")
    woT = nc.dram_tensor("woT", [128, NHC, H], DT.float16, kind="ExternalInput")
    cosT = nc.dram_tensor("cosT", [128, S], DT.float16, kind="ExternalInput")
    sinT = nc.dram_tensor("sinT", [128, S], DT.float16, kind="ExternalInput")
    trim = nc.dram_tensor("trim", [128, 128], DT.float16, kind="ExternalInput")
    outp = nc.dram_tensor("outp", [S, H], DT.float16, kind="ExternalOutput")

    with tile.TileContext(nc) as tc:
        with tc.tile_pool(name="persist", bufs=1) as persist, \
             tc.tile_pool(name="p2pt", bufs=8) as p2pt, \
             tc.tile_pool(name="p2acc", bufs=2) as p2acc, \
             tc.tile_pool(name="p2sum", bufs=2) as p2sum, \
             tc.tile_pool(name="p2y", bufs=2) as p2y:
            qT = persist.tile([128, NHC, S], DT.float16)    # [d, head, t]
            kT = persist.tile([128, NKVC, S], DT.float16)   # [d, kv, t]
            vsb = persist.tile([128, NT, NKVC * 128], DT.float16)
            biasc = persist.tile([128, 1], DT.float32)
            nc.vector.memset(biasc[:], EXP_BIAS)
            trimask = persist.tile([128, 128], DT.float16)
            nc.sync.dma_start(trimask[:], trim[:])
            wu = persist.tile([128, 512], DT.float16)
            nc.vector.memset(wu[:], 0.0)

            # PE p-state warm-up: dummy matmuls that run while the first
            # DMAs land, so real matmuls start at full clock.
            with tc.tile_pool(name="pwu", bufs=1, space="PSUM") as pwu:
                wups = pwu.tile([128, 512], DT.float32)
                for _ in range(14):
                    nc.tensor.matmul(wups[:], wu[:, 0:128], wu[:],
                                     start=True, stop=True)

            def attn_head(qj, h, yT, psS, psO, sdepth, post_pv):
                """Emit the full attention block for one (query strip, head)."""
                q0 = qj * 512
                nki = 4 * qj + 4
                kv = h // (NHC // NKVC)
                acc = p2acc.tile([128, 512], DT.float16, tag="acc",
                                 name="acc")
                ps_o = psO.tile([128, 512], DT.float32, tag="o", name="ps_o")
                ps_list = {}

                def emit_S(ki):
                    off = max(0, ki - 4 * qj) * 128
                    ps_s = psS.tile([128, 512], DT.float32, tag="s",
                                    name="ps_s")
                    nc.tensor.matmul(
                        ps_s[:, :512 - off],
                        kT[:, kv, 128 * ki:128 * ki + 128],
                        qT[:, h, q0 + off:q0 + 512],
                        start=True, stop=True)
                    ps_list[ki] = ps_s

                for ki in range(min(sdepth, nki)):
                    emit_S(ki)
                for ki in range(nki):
                    off = max(0, ki - 4 * qj) * 128
                    w_ = 512 - off
                    ps_s = ps_list.pop(ki)
                    pt = p2pt.tile([128, 512], DT.float16, tag="pt",
                                   name="pt")
                    nc.scalar.activation(pt[:, :w_], ps_s[:, :w_],
                                         ACTF.Exp, scale=SCALE,
                                         bias=biasc[:])
                    if ki >= 4 * qj:
                        nc.vector.tensor_mul(pt[:, 0:128], pt[:, 0:128],
                                             trimask[:])
                    if ki == 0:
                        nc.vector.tensor_copy(acc[:], pt[:])
                    else:
                        nc.vector.tensor_add(acc[:, off:], acc[:, off:],
                                             pt[:, :w_])
                    if ki + sdepth < nki:
                        emit_S(ki + sdepth)
                    nc.tensor.matmul(
                        ps_o[:, off:],
                        vsb[:, ki, 128 * kv:128 * kv + 128],
                        pt[:, :w_],
                        start=(ki == 0), stop=(ki == nki - 1),
                        skip_group_check=True)
                    if post_pv is not None:
                        post_pv()
                sums = p2sum.tile([128, 512], DT.float32, tag="sums",
                                  name="sums")
                nc.gpsimd.partition_all_reduce(
                    sums[:], acc[:], channels=128,
                    reduce_op=bass_isa.ReduceOp.add)
                rec = p2sum.tile([128, 512], DT.float32, tag="rec",
                                 name="rec")
                nc.vector.reciprocal(rec[:], sums[:])
                nc.vector.tensor_mul(yT[:, h, :], ps_o[:], rec[:])

            # ---------------- Phase 1: projections + rope + qj0 ----------
            with tc.tile_pool(name="p1c", bufs=1) as p1c, \
                 tc.tile_pool(name="p1x", bufs=2) as p1x, \
                 tc.tile_pool(name="p1w", bufs=2) as p1w, \
                 tc.tile_pool(name="p1ri", bufs=2) as p1ri, \
                 tc.tile_pool(name="p1t", bufs=2) as p1t, \
                 tc.tile_pool(name="p1ps", bufs=3, space="PSUM") as p1ps, \
                 tc.tile_pool(name="p1pv", bufs=2, space="PSUM") as p1pv, \
                 tc.tile_pool(name="psS0", bufs=2, space="PSUM") as psS0, \
                 tc.tile_pool(name="psO0", bufs=1, space="PSUM") as psO0:
                cos_sb = p1c.tile([128, S], DT.float16)
                sin_sb = p1c.tile([128, S], DT.float16)
                wv_sb = p1c.tile([128, KO, NKVC * 128], DT.float16)
                yT0 = p2y.tile([128, NHC, 512], DT.float16, tag="y",
                               name="yT0")

                def rope(dst, ri, t0):
                    # cos/sin halves are identical (emb = concat(freqs,
                    # freqs)), so each product reads the half that matches
                    # its ri operand's base partition (BIR requires equal
                    # SBUF base partitions for DVE inputs).
                    cl = cos_sb[0:64, t0:t0 + 512]
                    sl = sin_sb[0:64, t0:t0 + 512]
                    ch = cos_sb[64:128, t0:t0 + 512]
                    sh = sin_sb[64:128, t0:t0 + 512]
                    t1 = p1t.tile([64, 512], DT.float16, tag="t1")
                    t2 = p1t.tile([64, 512], DT.float16, tag="t2")
                    nc.vector.tensor_mul(t1[:], ri[0:64, :], cl)
                    nc.vector.tensor_mul(t2[:], ri[64:128, :], sh)
                    nc.vector.tensor_sub(dst[0:64, :], t1[:], t2[:])
                    t3 = p1t.tile([64, 512], DT.float16, tag="t3")
                    t4 = p1t.tile([64, 512], DT.float16, tag="t4")
                    nc.vector.tensor_mul(t3[:], ri[64:128, :], ch)
                    nc.vector.tensor_mul(t4[:], ri[0:64, :], sl)
                    nc.vector.tensor_add(dst[64:128, :], t3[:], t4[:])

                qj0_next = [0]

                def qj0_step():
                    if qj0_next[0] < NHC:
                        attn_head(0, qj0_next[0], yT0, psS0, psO0, 2, None)
                        qj0_next[0] += 1

                slot = 0
                for ts in range(4):
                    t0 = ts * 512
                    xs = p1x.tile([128, KO, 512], DT.float16, tag="xs")
                    for c in range(4):
                        nc.sync.dma_start(xs[:, 8 * c:8 * c + 8, :],
                                          xT[:, 8 * c:8 * c + 8, t0:t0 + 512])
                    if ts == 0:
                        nc.sync.dma_start(cos_sb[:], cosT[:])
                        nc.sync.dma_start(sin_sb[:], sinT[:])
                        nc.sync.dma_start(wv_sb[:], wvT[:])
                    for h in range(NPROJ):
                        wt = p1w.tile([128, KO, 128], DT.float16, tag="w")
                        nc.sync.dma_start(wt[:], wqk[h])
                        ps = p1ps.tile([128, 512], DT.float32, tag="p")
                        for ko in range(KO):
                            nc.tensor.matmul(ps[:], wt[:, ko, :], xs[:, ko, :],
                                             start=(ko == 0),
                                             stop=(ko == KO - 1))
                        ri = p1ri.tile([128, 512], DT.float16, tag="ri")
                        nc.scalar.copy(ri[:], ps[:])
                        if h < NHC:
                            rope(qT[:, h, t0:t0 + 512], ri, t0)
                        else:
                            rope(kT[:, h - NHC, t0:t0 + 512], ri, t0)
                        if ts >= 1:
                            if slot % 2 == 0:
                                qj0_step()
                            slot += 1
                    for mt in range(4):
                        psv = p1pv.tile([128, NKVC * 128], DT.float32,
                                        tag="pv")
                        for ko in range(KO):
                            nc.tensor.matmul(
                                psv[:], xs[:, ko, mt * 128:(mt + 1) * 128],
                                wv_sb[:, ko, :],
                                start=(ko == 0), stop=(ko == KO - 1))
                        nc.scalar.copy(vsb[:, ts * 4 + mt, :], psv[:])
                while qj0_next[0] < NHC:
                    qj0_step()

            # ------- Phases 2+3 interleaved (strips 1-3) -------
            with tc.tile_pool(name="p2c", bufs=1) as p2c, \
                 tc.tile_pool(name="p3o", bufs=3) as p3o, \
                 tc.tile_pool(name="psS", bufs=3, space="PSUM") as psS, \
                 tc.tile_pool(name="psO", bufs=3, space="PSUM") as psO, \
                 tc.tile_pool(name="p3ps", bufs=2, space="PSUM") as p3ps:
                wo_sb = p2c.tile([128, NHC, H], DT.float16)
                for ntc in range(8):
                    nc.sync.dma_start(
                        wo_sb[:, :, 512 * ntc:512 * ntc + 512],
                        woT[:, :, 512 * ntc:512 * ntc + 512])

                # o_proj filler machinery: one step = one PE matmul of the
                # previous strip's o_proj, emitted between attention matmuls.
                fst = {"st": 0, "yT": yT0, "cur": None,
                       "items": [(tt, 512 * nt)
                                 for tt in range(4) for nt in range(8)]}

                def filler_step():
                    if fst["cur"] is None:
                        if not fst["items"]:
                            return False
                        tt, n0 = fst["items"].pop(0)
                        ps3 = p3ps.tile([128, 512], DT.float32, tag="p3",
                                        name="ps3")
                        fst["cur"] = [tt, n0, 0, ps3]
                    tt, n0, hh, ps3 = fst["cur"]
                    nc.tensor.matmul(ps3[:],
                                     fst["yT"][:, hh, 128 * tt:128 * tt + 128],
                                     wo_sb[:, hh, n0:n0 + 512],
                                     start=(hh == 0), stop=(hh == NHC - 1))
                    if hh == NHC - 1:
                        ot = p3o.tile([128, 512], DT.float16, tag="ot",
                                      name="ot")
                        nc.scalar.copy(ot[:], ps3[:])
                        row0 = (4 * fst["st"] + tt) * 128
                        nc.sync.dma_start(outp[row0:row0 + 128, n0:n0 + 512],
                                          ot[:])
                        fst["cur"] = None
                    else:
                        fst["cur"][2] = hh + 1
                    return True

                def post_pv():
                    filler_step()
                    filler_step()

                for qj in range(1, 4):
                    yT = p2y.tile([128, NHC, 512], DT.float16, tag="y",
                                  name="yT")
                    for h in range(NHC):
                        attn_head(qj, h, yT, psS, psO, 3, post_pv)
                    while filler_step():
                        pass
                    fst["st"] = qj
                    fst["yT"] = yT
                    fst["items"] = [(tt, 512 * nt)
                                    for tt in range(4) for nt in range(8)]
                while filler_step():
                    pass
    nc.compile()
    return nc


def _get_nc():
    if "nc" not in _CACHE:
        _CACHE["nc"] = _build_nc()
    return _CACHE["nc"]


def _host_prep(hidden_states, position_ids, wq, wk, wv, wo):
    """Build the 8 per-core input maps (all fp16, contiguity-packed)."""
    F16 = np.float16
    inv_freq = 1.0 / (ROPE_THETA ** (np.arange(0, D, 2, dtype=np.float32) / D))
    p = np.arange(128)[:, None]
    f = np.arange(128)[None, :]
    trim = (p <= f).astype(F16)

    DQ = NHC * D
    DKV = NKVC * D
    wq = np.asarray(wq, dtype=np.float32)
    wk = np.asarray(wk, dtype=np.float32)
    wv = np.asarray(wv, dtype=np.float32)
    wo = np.asarray(wo, dtype=np.float32)

    in_maps = []
    for c in range(8):
        b, g = divmod(c, TP)
        x = np.asarray(hidden_states[b], dtype=np.float32)
        xTa = np.ascontiguousarray(
            x.T.reshape(KO, 128, S).transpose(1, 0, 2)).astype(F16)
        wq_g = wq[DQ * g:DQ * (g + 1)].reshape(NHC, 128, KO, 128)
        wk_g = wk[DKV * g:DKV * (g + 1)].reshape(NKVC, 128, KO, 128)
        wqk_a = np.ascontiguousarray(
            np.concatenate([wq_g, wk_g], axis=0).transpose(0, 3, 2, 1)
        ).astype(F16)
        wvT_a = np.ascontiguousarray(
            wv[DKV * g:DKV * (g + 1)].reshape(DKV, KO, 128).transpose(2, 1, 0)
        ).astype(F16)
        woT_a = np.ascontiguousarray(
            wo[:, DQ * g:DQ * (g + 1)].T.reshape(NHC, 128, H).transpose(1, 0, 2)
        ).astype(F16)
        pos = np.asarray(position_ids[b], dtype=np.float32)
        freqs = pos[:, None] * inv_freq[None, :]            # [S, 64]
        emb = np.concatenate([freqs, freqs], axis=1)        # [S, 128]
        cosT_a = np.ascontiguousarray(np.cos(emb).T).astype(F16)
        sinT_a = np.ascontiguousarray(np.sin(emb).T).astype(F16)
        in_maps.append({
            "xT": xTa, "wqk": wqk_a, "wvT": wvT_a, "woT": woT_a,
            "cosT": cosT_a, "sinT": sinT_a, "trim": trim,
        })
    return in_maps


def kernel(hidden_states, position_ids, wq, wk, wv, wo):
    from concourse.bass_utils import run_bass_kernel_spmd

    hidden_states = np.asarray(hidden_states)
    in_maps = _host_prep(hidden_states, position_ids, wq, wk, wv, wo)
    nc = _get_nc()
    res = run_bass_kernel_spmd(nc, in_maps, list(range(8)))
    out = np.zeros((DP, S, H), dtype=np.float32)
    for c in range(8):
        b = c // TP
        out[b] += res.results[c]["outp"].astype(np.float32)
    return out
